# revision 3
# baseline (speedup 1.0000x reference)
"""DenseSigmoidFlow Trainium2 kernel.

Math (exact algebraic reduction of the reference):
  per (b, n):
    a[h]  = softplus(ds0[h] + inv) + EPS
    ed[i] = exp(ds3[i]);  ee[h] = exp(ds2[h])
    Su[h] = sum_i exp(u_[h,i]) * ed[i]          (softmax denominators)
    Tn[h] = sum_i exp(u_[h,i]) * ed[i] * x[i]
    pre   = a * Tn/Su + ds1
    sigm  = 1/(1+exp(-pre))
    es    = a * sigm * (1-sigm)
    Sw[o] = sum_h exp(w_[o,h]) * ee[h]
    Xn[o] = sum_h exp(w_[o,h]) * ee[h] * sigm[h]
    Rn[o] = sum_h exp(w_[o,h]) * ee[h] * es[h]
    xpre  = Xn/Sw
    lx    = log((1-EPS)*xpre + EPS/2);  l1mx = log(1 - EPS/2 - (1-EPS)*xpre)
    xnew[o]   = (lx - l1mx)            (n = 0 only)
    ldout[o]  = log(Rn/Sw) - lx - l1mx + log(1-EPS) + 2*EPS + logdet[b]

The 5-D logsumexp in the reference collapses because all terms are positive
and softmax rows sum to one, leaving three 16x16 matvecs per (b, n) against
the shared matrices exp(u_), exp(w_) - done as block-diagonal 128x128
matmuls on the PE array with (n, dim) packed on partitions and the batch on
the free dimension.

Sharding: batch dim B=4096 split across 8 cores (512 rows each).
"""

import math

import numpy as np

B, N, H, IN, OUT = 4096, 8, 16, 16, 16
NCORES = 8
BC = B // NCORES  # 512 batch rows per core
F = BC            # free dim per core
CHUNK = 256
EPS = 1e-6
INV = math.log(math.exp(1.0 - EPS) - 1.0)
C_LD = math.log(1.0 - EPS) + 2.0 * EPS  # fold of log(1-EPS) and the 2*EPS from logsigmoid

_cache = {}
RUN_KWARGS = {}  # test harness may set {"trace": True}


def _build():
    import concourse.bacc as bacc
    import concourse.tile as tile
    from concourse import mybir

    f32 = mybir.dt.float32
    AF = mybir.ActivationFunctionType

    nc = bacc.Bacc("TRN2", target_bir_lowering=False, debug=False)

    # inputs ([p, ...]: partition p = n*16 + d, free = batch-local b)
    dsp = nc.dram_tensor("dsp", [128, 4 * F], f32, kind="ExternalInput").ap()
    xrep = nc.dram_tensor("xrep", [128, F], f32, kind="ExternalInput").ap()
    ldb = nc.dram_tensor("ldb", [128, F], f32, kind="ExternalInput").ap()
    eut = nc.dram_tensor("eut", [128, 128], f32, kind="ExternalInput").ap()
    ewt = nc.dram_tensor("ewt", [128, 128], f32, kind="ExternalInput").ap()
    # outputs
    xn_d = nc.dram_tensor("xn", [16, F], f32, kind="ExternalOutput").ap()
    ld_d = nc.dram_tensor("ld", [128, F], f32, kind="ExternalOutput").ap()

    with tile.TileContext(nc) as tc:
        import contextlib

        with contextlib.ExitStack() as ctx:
            consts = ctx.enter_context(tc.tile_pool(name="consts", bufs=1))
            wpool = ctx.enter_context(tc.tile_pool(name="weights", bufs=1))
            inp = ctx.enter_context(tc.tile_pool(name="inp", bufs=3))
            work = ctx.enter_context(tc.tile_pool(name="work", bufs=3))
            outp = ctx.enter_context(tc.tile_pool(name="outp", bufs=3))
            psum = ctx.enter_context(tc.tile_pool(name="psum", bufs=1, space="PSUM"))

            # per-partition bias constants for ACT (bias must be an AP)
            def bias_const(name, val):
                t = consts.tile([128, 1], f32, tag=name)
                nc.gpsimd.memset(t[:], val)
                return t

            inv_b = bias_const("inv", INV)
            one_b = bias_const("one", 1.0)
            he_b = bias_const("he", EPS / 2)
            ohe_b = bias_const("ohe", 1.0 - EPS / 2)

            # stationary block-diagonal weights
            eut_s = wpool.tile([128, 128], f32, tag="eut")
            nc.sync.dma_start(eut_s[:], eut[:])
            ewt_s = wpool.tile([128, 128], f32, tag="ewt")
            nc.sync.dma_start(ewt_s[:], ewt[:])
            # whole-core broadcast inputs
            xrep_s = wpool.tile([128, F], f32, tag="xrep")
            nc.sync.dma_start(xrep_s[:], xrep[:])
            ldb_s = wpool.tile([128, F], f32, tag="ldb")
            nc.sync.dma_start(ldb_s[:], ldb[:])

            for c in range(F // CHUNK):
                c0, c1 = c * CHUNK, (c + 1) * CHUNK

                # field loads (fields laid out at free offset f*F + b)
                A0 = inp.tile([128, CHUNK], f32, tag="A0")
                nc.sync.dma_start(A0[:], dsp[:, 0 * F + c0 : 0 * F + c1])
                A1 = inp.tile([128, CHUNK], f32, tag="A1")
                nc.sync.dma_start(A1[:], dsp[:, 1 * F + c0 : 1 * F + c1])
                A2 = inp.tile([128, CHUNK], f32, tag="A2")
                nc.sync.dma_start(A2[:], dsp[:, 2 * F + c0 : 2 * F + c1])
                A3 = inp.tile([128, CHUNK], f32, tag="A3")
                nc.sync.dma_start(A3[:], dsp[:, 3 * F + c0 : 3 * F + c1])

                # a = softplus(A0 + inv) + EPS
                ea = work.tile([128, CHUNK], f32, tag="ea")
                nc.scalar.activation(ea[:], A0[:], AF.Exp, bias=inv_b[:])
                spl = work.tile([128, CHUNK], f32, tag="spl")
                nc.scalar.activation(spl[:], ea[:], AF.Ln, bias=one_b[:])
                a_t = work.tile([128, CHUNK], f32, tag="a_t")
                nc.vector.tensor_scalar_add(a_t[:], spl[:], EPS)

                # ed, ee, edx
                ed = work.tile([128, CHUNK], f32, tag="ed")
                nc.scalar.activation(ed[:], A3[:], AF.Exp)
                ee = work.tile([128, CHUNK], f32, tag="ee")
                nc.scalar.activation(ee[:], A2[:], AF.Exp)
                edx = work.tile([128, CHUNK], f32, tag="edx")
                nc.vector.tensor_mul(edx[:], ed[:], xrep_s[:, c0:c1])

                # u-side matmuls
                su_p = psum.tile([128, CHUNK], f32, tag="su")
                nc.tensor.matmul(su_p[:], eut_s[:], ed[:], start=True, stop=True)
                tn_p = psum.tile([128, CHUNK], f32, tag="tn")
                nc.tensor.matmul(tn_p[:], eut_s[:], edx[:], start=True, stop=True)

                # pre = a * tn/su + A1 ; te = exp(-pre)
                rsu = work.tile([128, CHUNK], f32, tag="rsu")
                nc.vector.reciprocal(rsu[:], su_p[:])
                tt = work.tile([128, CHUNK], f32, tag="tt")
                nc.vector.tensor_mul(tt[:], tn_p[:], rsu[:])
                tt2 = work.tile([128, CHUNK], f32, tag="tt2")
                nc.vector.tensor_mul(tt2[:], tt[:], a_t[:])
                pre = work.tile([128, CHUNK], f32, tag="pre")
                nc.vector.tensor_add(pre[:], tt2[:], A1[:])
                te = work.tile([128, CHUNK], f32, tag="te")
                nc.scalar.activation(te[:], pre[:], AF.Exp, scale=-1.0)

                # sigm = 1/(1+te); es = a * te * sigm^2
                u1 = work.tile([128, CHUNK], f32, tag="u1")
                nc.vector.tensor_scalar_add(u1[:], te[:], 1.0)
                r1 = work.tile([128, CHUNK], f32, tag="r1")
                nc.vector.reciprocal(r1[:], u1[:])
                m1 = work.tile([128, CHUNK], f32, tag="m1")
                nc.vector.tensor_mul(m1[:], te[:], r1[:])
                m2 = work.tile([128, CHUNK], f32, tag="m2")
                nc.vector.tensor_mul(m2[:], m1[:], r1[:])
                es = work.tile([128, CHUNK], f32, tag="es")
                nc.vector.tensor_mul(es[:], m2[:], a_t[:])

                # w-side rhs
                esig = work.tile([128, CHUNK], f32, tag="esig")
                nc.vector.tensor_mul(esig[:], ee[:], r1[:])
                ees = work.tile([128, CHUNK], f32, tag="ees")
                nc.vector.tensor_mul(ees[:], ee[:], es[:])

                # w-side matmuls
                sw_p = psum.tile([128, CHUNK], f32, tag="sw")
                nc.tensor.matmul(sw_p[:], ewt_s[:], ee[:], start=True, stop=True)
                xn_p = psum.tile([128, CHUNK], f32, tag="xnm")
                nc.tensor.matmul(xn_p[:], ewt_s[:], esig[:], start=True, stop=True)
                rn_p = psum.tile([128, CHUNK], f32, tag="rn")
                nc.tensor.matmul(rn_p[:], ewt_s[:], ees[:], start=True, stop=True)

                # xpre = Xn/Sw ; r = Rn/Sw
                rsw = work.tile([128, CHUNK], f32, tag="rsw")
                nc.vector.reciprocal(rsw[:], sw_p[:])
                xpre = work.tile([128, CHUNK], f32, tag="xpre")
                nc.vector.tensor_mul(xpre[:], xn_p[:], rsw[:])
                rr = work.tile([128, CHUNK], f32, tag="rr")
                nc.vector.tensor_mul(rr[:], rn_p[:], rsw[:])

                # logs
                lx = work.tile([128, CHUNK], f32, tag="lx")
                nc.scalar.activation(lx[:], xpre[:], AF.Ln, bias=he_b[:], scale=1.0 - EPS)
                l1mx = work.tile([128, CHUNK], f32, tag="l1mx")
                nc.scalar.activation(
                    l1mx[:], xpre[:], AF.Ln, bias=ohe_b[:], scale=-(1.0 - EPS)
                )
                logr = work.tile([128, CHUNK], f32, tag="logr")
                nc.scalar.activation(logr[:], rr[:], AF.Ln)

                # ldout = logr - lx - l1mx + (C + logdet)
                s1 = work.tile([128, CHUNK], f32, tag="s1")
                nc.vector.tensor_sub(s1[:], logr[:], lx[:])
                s2 = work.tile([128, CHUNK], f32, tag="s2")
                nc.vector.tensor_sub(s2[:], s1[:], l1mx[:])
                ldo = outp.tile([128, CHUNK], f32, tag="ldo")
                nc.vector.tensor_add(ldo[:], s2[:], ldb_s[:, c0:c1])
                nc.sync.dma_start(ld_d[:, c0:c1], ldo[:])

                # xnew (n = 0 partitions only)
                xno = outp.tile([16, CHUNK], f32, tag="xno")
                nc.vector.tensor_sub(xno[:], lx[0:16, :], l1mx[0:16, :])
                nc.sync.dma_start(xn_d[:, c0:c1], xno[:])

    nc.compile()
    return nc


def _get_nc():
    if "nc" not in _cache:
        _cache["nc"] = _build()
    return _cache["nc"]


def kernel(dsparams, x, logdet, u_, w_):
    from concourse.bass_utils import run_bass_kernel_spmd

    dsparams = np.ascontiguousarray(dsparams, dtype=np.float32)
    x = np.ascontiguousarray(x, dtype=np.float32)
    logdet = np.ascontiguousarray(logdet, dtype=np.float32)
    u_ = np.ascontiguousarray(u_, dtype=np.float32)
    w_ = np.ascontiguousarray(w_, dtype=np.float32)

    # --- host-side sharding / layout prep ---
    # dsp[c, n*16+d, f*F+b] = dsparams[c*BC+b, n, f*16+d]
    dsr = dsparams.reshape(NCORES, BC, N, 4, 16)
    dsp = np.ascontiguousarray(dsr.transpose(0, 2, 4, 3, 1)).reshape(NCORES, 128, 4 * F)

    # xrep[c, n*16+i, b] = x[c*BC+b, i]
    xc = x.reshape(NCORES, BC, IN).transpose(0, 2, 1)  # [8, 16, BC]
    xrep = np.ascontiguousarray(
        np.broadcast_to(xc[:, None, :, :], (NCORES, N, IN, BC))
    ).reshape(NCORES, 128, F)

    # ldb[c, p, b] = logdet[c*BC+b] + C_LD
    ldc = logdet.reshape(NCORES, BC).astype(np.float32) + np.float32(C_LD)
    ldb = np.ascontiguousarray(
        np.broadcast_to(ldc[:, None, :], (NCORES, 128, F))
    ).astype(np.float32)

    # block-diagonal stationary matrices (tiny parameter transform)
    eu_t = np.exp(u_).T  # [i, h]
    ew_t = np.exp(w_).T  # [h, o]
    eut = np.zeros((128, 128), np.float32)
    ewt = np.zeros((128, 128), np.float32)
    for g in range(8):
        eut[16 * g : 16 * g + 16, 16 * g : 16 * g + 16] = eu_t
        ewt[16 * g : 16 * g + 16, 16 * g : 16 * g + 16] = ew_t

    nc = _get_nc()
    in_maps = [
        {
            "dsp": np.ascontiguousarray(dsp[c]),
            "xrep": np.ascontiguousarray(xrep[c]),
            "ldb": np.ascontiguousarray(ldb[c]),
            "eut": eut,
            "ewt": ewt,
        }
        for c in range(NCORES)
    ]
    res = run_bass_kernel_spmd(nc, in_maps, core_ids=list(range(NCORES)), **RUN_KWARGS)
    _cache["last_result"] = res

    # --- gather ---
    xnew = np.empty((B, OUT), np.float32)
    ldout = np.empty((B, N, OUT, 1), np.float32)
    for c in range(NCORES):
        xn = res.results[c]["xn"]  # [16, BC]
        ld = res.results[c]["ld"]  # [128, BC]
        xnew[c * BC : (c + 1) * BC, :] = xn.T
        ldout[c * BC : (c + 1) * BC] = ld.reshape(N, OUT, BC).transpose(2, 0, 1)[
            :, :, :, None
        ]
    return xnew, ldout
